# revision 11
# baseline (speedup 1.0000x reference)
"""Mixture-of-Experts (8 experts, top-2, D=1024, H=2048, T=8192) on 8 trn2 cores.

Strategy: expert-parallel with host-side routing.
  - Router (tiny: [T,D]@[D,E]) runs on host in float64; top-2 selection was
    verified to match fp32 jax (cpu + neuron) selection for this problem size.
  - Each core owns one expert and computes SwiGLU on only the tokens routed
    to it (~T*2/E rows instead of T), padded to capacity C.
  - Activations flow in transposed (feature-major) layout so the kernel needs
    no on-device transposes:
        h1T = w1 @ xT   (accumulate over D chunks)   [H, C]
        hT  = silu(h1T) * h3T                        [H, C]  (bf16)
        y   = (hT.T chunks) @ w2T                    [C, D]  (tokens on
              partitions so the per-token combine-weight scale is a
              per-partition tensor_scalar op)
  - Host combines: out[t] = y_e1[slot1] + y_e2[slot2] (cw applied on device).
"""

import sys
import types
from contextlib import ExitStack

import ml_dtypes
import numpy as np

import concourse.bass as bass
import concourse.tile as tile
from concourse import bacc, mybir
from concourse.bass_utils import run_bass_kernel_spmd


def install_axon_hooks_shim():
    """The container's antenv stub lacks axon_hooks, which
    run_bass_kernel_spmd imports whenever tracing is requested (including
    via the BASS_TRACE env var). Recreate it and register the NTFF
    profiling hook if the axon PJRT .so is present."""
    try:
        import antenv
    except ImportError:
        return False
    if "antenv.axon_hooks" in sys.modules:
        return sys.modules["antenv.axon_hooks"]._hook is not None
    mod = types.ModuleType("antenv.axon_hooks")
    mod._hook = None
    mod.set_axon_ntff_profile_hook = lambda h: setattr(mod, "_hook", h)
    mod.get_axon_ntff_profile_hook = lambda: mod._hook
    sys.modules["antenv.axon_hooks"] = mod
    antenv.axon_hooks = mod
    try:
        from trn_agent_boot.trn_boot import _ntff_profile_via_ctypes

        mod.set_axon_ntff_profile_hook(
            _ntff_profile_via_ctypes("/opt/axon/libaxon_pjrt.so")
        )
    except Exception:
        pass
    return mod._hook is not None


install_axon_hooks_shim()

E = 8  # experts == cores
D = 1024
H = 2048
TOP_K = 2

BF16 = mybir.dt.bfloat16
F32 = mybir.dt.float32

_CACHE: dict[int, object] = {}


def _route(x2d: np.ndarray, router_w: np.ndarray):
    """Float64 router. Returns per-expert token lists, per-expert combine
    weights, and for each token its (expert, slot-in-expert-batch) pairs."""
    T = x2d.shape[0]
    logits = x2d.astype(np.float64) @ router_w.astype(np.float64).T  # [T, E]
    order = np.argsort(-logits, axis=1, kind="stable")
    top2 = order[:, :TOP_K]  # [T, 2]
    lt = np.take_along_axis(logits, top2, axis=1)
    m = lt.max(axis=1, keepdims=True)
    ex = np.exp(lt - m)
    cw = (ex / ex.sum(axis=1, keepdims=True)).astype(np.float32)  # [T, 2]

    rows = []  # rows[e]: token ids routed to expert e (ascending)
    cw_e = []  # cw_e[e]: combine weight per routed token
    slot = np.empty((T, TOP_K), np.int64)  # slot[t, k]: row of t in expert batch
    for e in range(E):
        r = np.where((top2[:, 0] == e) | (top2[:, 1] == e))[0]
        k = np.where(top2[r, 0] == e, 0, 1)
        rows.append(r)
        cw_e.append(cw[r, k])
        slot[r, k] = np.arange(len(r))
    return rows, cw_e, top2, slot


def _build(C: int):
    """Build + compile the per-core Bass program for capacity C (mult of 128)."""
    assert C % 128 == 0
    nsub = C // 128  # token subtiles
    nc = bacc.Bacc("TRN2", target_bir_lowering=False, debug=False)

    xt = nc.declare_dram_parameter("xt", [D, C], BF16, isOutput=False)
    w1t = nc.declare_dram_parameter("w1t", [D, H], BF16, isOutput=False)
    w3t = nc.declare_dram_parameter("w3t", [D, H], BF16, isOutput=False)
    w2t = nc.declare_dram_parameter("w2t", [H, D], BF16, isOutput=False)
    cwt = nc.declare_dram_parameter("cwt", [128, nsub], F32, isOutput=False)
    y = nc.declare_dram_parameter("y", [C, D], F32, isOutput=True)

    xt_v = xt.rearrange("(a p) c -> p a c", p=128)  # [128, 8, C]
    w1_v = w1t.rearrange("(a p) h -> p a h", p=128)  # [128, 8, H]
    w3_v = w3t.rearrange("(a p) h -> p a h", p=128)
    w2_v = w2t.rearrange("(m p) d -> p m d", p=128)  # [128, 16, D]
    y_v = y.rearrange("(n p) d -> n p d", p=128)  # [nsub, 128, D]

    KA = D // 128  # 8 contraction chunks for matmul 1
    KM = H // 128  # 16 contraction chunks for matmul 2

    # token blocks: small first block (cheap DMA gate for the first matmul),
    # then 512s + remainder (all multiples of 128)
    blocks = [(0, min(128, C))]
    t0 = blocks[0][1]
    while t0 < C:
        tb = min(512, C - t0)
        blocks.append((t0, tb))
        t0 += tb

    with ExitStack() as ctx:
        tc = ctx.enter_context(tile.TileContext(nc))
        wpool = ctx.enter_context(tc.tile_pool(name="weights", bufs=1))
        xpool = ctx.enter_context(tc.tile_pool(name="x", bufs=2))
        hpool = ctx.enter_context(tc.tile_pool(name="h", bufs=2))
        spool = ctx.enter_context(tc.tile_pool(name="s", bufs=3))
        ypool = ctx.enter_context(tc.tile_pool(name="y", bufs=4))
        ppool = ctx.enter_context(tc.tile_pool(name="psum", bufs=2, space="PSUM"))

        # first token block's activations first — they gate the first matmul
        def xts_load(t0, tb):
            xa = xpool.tile([128, KA, tb], BF16, tag="xts")
            nc.sync.dma_start(xa[:], xt_v[:, :, t0 : t0 + tb])
            return xa

        xts0 = xts_load(0, blocks[0][1])

        # weights split into pieces (small first) so early matmuls start early
        PIECES = (1, 1, 2, 4, 4, 4)  # m-chunks per piece, sums to KM
        assert sum(PIECES) == KM
        w1p, w3p = [], []  # per m-chunk: (tile, offset)
        m0 = 0
        for p, sz in enumerate(PIECES):
            hs = slice(m0 * 128, (m0 + sz) * 128)
            t1 = wpool.tile([128, KA, sz * 128], BF16, tag=f"w1s{p}")
            nc.sync.dma_start(t1[:], w1_v[:, :, hs])
            t3 = wpool.tile([128, KA, sz * 128], BF16, tag=f"w3s{p}")
            nc.sync.dma_start(t3[:], w3_v[:, :, hs])
            for i in range(sz):
                w1p.append((t1, i))
                w3p.append((t3, i))
            m0 += sz

        w2p = []  # [m-half][d-half] tiles of [128, KM//2, 512]
        for mh in range(2):
            msl = slice(mh * (KM // 2), (mh + 1) * (KM // 2))
            row = []
            for dh in range(2):
                dsl = slice(dh * 512, (dh + 1) * 512)
                t2 = wpool.tile([128, KM // 2, 512], BF16, tag=f"w2s{mh}{dh}")
                nc.sync.dma_start(t2[:], w2_v[:, msl, dsl])
                row.append(t2)
            w2p.append(row)
        cws = wpool.tile([128, nsub], F32, tag="cws")
        nc.sync.dma_start(cws[:], cwt[:])

        for bi, (t0, tb) in enumerate(blocks):
            xts = xts0 if bi == 0 else xts_load(t0, tb)

            hts = hpool.tile([128, KM, tb], BF16, tag="hts")

            # phase A: h1T/h3T chunks + silu * mul -> hts
            for m in range(KM):
                w1s, o1 = w1p[m]
                w3s, o3 = w3p[m]
                ph1 = ppool.tile([128, tb], F32, tag="ph1")
                for a in range(KA):
                    nc.tensor.matmul(
                        ph1[:],
                        w1s[:, a, bass.ts(o1, 128)],
                        xts[:, a, :],
                        start=(a == 0),
                        stop=(a == KA - 1),
                    )
                ph3 = ppool.tile([128, tb], F32, tag="ph3")
                for a in range(KA):
                    nc.tensor.matmul(
                        ph3[:],
                        w3s[:, a, bass.ts(o3, 128)],
                        xts[:, a, :],
                        start=(a == 0),
                        stop=(a == KA - 1),
                    )
                sil = spool.tile([128, tb], BF16, tag="sil")
                nc.scalar.activation(
                    sil[:], ph1[:], mybir.ActivationFunctionType.Silu
                )
                nc.vector.tensor_mul(hts[:, m, :], sil[:], ph3[:])

            # phase B: y = hT.T @ w2T, scaled by cw
            for n in range(tb // 128):
                nsl = bass.ts(n, 128)
                gn = t0 // 128 + n  # global subtile index
                py0 = ppool.tile([128, 512], F32, tag="py0")
                py1 = ppool.tile([128, 512], F32, tag="py1")
                for m in range(KM):
                    mh, mr = divmod(m, KM // 2)
                    nc.tensor.matmul(
                        py0[:],
                        hts[:, m, nsl],
                        w2p[mh][0][:, mr, :],
                        start=(m == 0),
                        stop=(m == KM - 1),
                    )
                    nc.tensor.matmul(
                        py1[:],
                        hts[:, m, nsl],
                        w2p[mh][1][:, mr, :],
                        start=(m == 0),
                        stop=(m == KM - 1),
                    )
                ys0 = ypool.tile([128, 512], F32, tag="ys0")
                nc.vector.tensor_scalar_mul(ys0[:], py0[:], cws[:, gn : gn + 1])
                nc.sync.dma_start(y_v[gn][:, 0:512], ys0[:])
                ys1 = ypool.tile([128, 512], F32, tag="ys1")
                nc.vector.tensor_scalar_mul(ys1[:], py1[:], cws[:, gn : gn + 1])
                nc.sync.dma_start(y_v[gn][:, 512:1024], ys1[:])

    nc.compile()
    return nc


def _get(C: int):
    if C not in _CACHE:
        _CACHE[C] = _build(C)
    return _CACHE[C]


def _prepare_core_inputs(x2d, w1, w2, w3, rows, cw_e, C):
    bf = ml_dtypes.bfloat16
    nsub = C // 128
    in_maps = []
    for e in range(E):
        ce = len(rows[e])
        xt = np.zeros((D, C), bf)
        xt[:, :ce] = x2d[rows[e]].T.astype(bf)
        cwt = np.zeros((C,), np.float32)
        cwt[:ce] = cw_e[e]
        in_maps.append(
            {
                "xt": xt,
                "w1t": np.ascontiguousarray(w1[e].T.astype(bf)),  # [D, H]
                "w3t": np.ascontiguousarray(w3[e].T.astype(bf)),  # [D, H]
                "w2t": np.ascontiguousarray(w2[e].T.astype(bf)),  # [H, D]
                "cwt": np.ascontiguousarray(cwt.reshape(nsub, 128).T),
            }
        )
    return in_maps


def run(inputs: dict, trace: bool = False, trace_cores=None):
    """Core implementation; returns (output, BassKernelResults)."""
    x = np.asarray(inputs["x"])
    router_w = np.asarray(inputs["router_w"], np.float32)
    w1 = np.asarray(inputs["w1"], np.float32)
    w2 = np.asarray(inputs["w2"], np.float32)
    w3 = np.asarray(inputs["w3"], np.float32)

    B, S, _ = x.shape
    x2d = np.ascontiguousarray(x.reshape(-1, D).astype(np.float32))
    T = x2d.shape[0]

    rows, cw_e, top2, slot = _route(x2d, router_w)
    cmax = max(len(r) for r in rows)
    C = max(128, int(np.ceil(cmax / 128) * 128))

    nc = _get(C)
    in_maps = _prepare_core_inputs(x2d, w1, w2, w3, rows, cw_e, C)
    res = run_bass_kernel_spmd(
        nc,
        in_maps,
        list(range(E)),
        trace=trace,
        trace_cores=trace_cores,
    )

    Y = np.stack([res.results[e]["y"] for e in range(E)])  # [E, C, D] f32
    Yf = Y.reshape(E * C, D)
    fi = top2.astype(np.int64) * C + slot  # [T, 2]
    out = Yf[fi[:, 0]] + Yf[fi[:, 1]]
    return out.reshape(B, S, D).astype(x.dtype), res


def kernel(**inputs) -> np.ndarray:
    out, _ = run(inputs, trace=False)
    return out


# revision 12
# speedup vs baseline: 1.0460x; 1.0460x over previous
"""Mixture-of-Experts (8 experts, top-2, D=1024, H=2048, T=8192) on 8 trn2 cores.

Strategy: expert-parallel with host-side routing.
  - Router (tiny: [T,D]@[D,E]) runs on host in float64; top-2 selection was
    verified to match fp32 jax (cpu + neuron) selection for this problem size.
  - Each core owns one expert and computes SwiGLU on only the tokens routed
    to it (~T*2/E rows instead of T), padded to capacity C.
  - Activations flow in transposed (feature-major) layout so the kernel needs
    no on-device transposes:
        h1T = w1 @ xT   (accumulate over D chunks)   [H, C]
        hT  = silu(h1T) * h3T                        [H, C]  (bf16)
        y   = (hT.T chunks) @ w2T                    [C, D]  (tokens on
              partitions so the per-token combine-weight scale is a
              per-partition tensor_scalar op)
  - Host combines: out[t] = y_e1[slot1] + y_e2[slot2] (cw applied on device).
"""

import sys
import types
from contextlib import ExitStack

import ml_dtypes
import numpy as np

import concourse.bass as bass
import concourse.tile as tile
from concourse import bacc, mybir
from concourse.bass_utils import run_bass_kernel_spmd


def install_axon_hooks_shim():
    """The container's antenv stub lacks axon_hooks, which
    run_bass_kernel_spmd imports whenever tracing is requested (including
    via the BASS_TRACE env var). Recreate it and register the NTFF
    profiling hook if the axon PJRT .so is present."""
    try:
        import antenv
    except ImportError:
        return False
    if "antenv.axon_hooks" in sys.modules:
        return sys.modules["antenv.axon_hooks"]._hook is not None
    mod = types.ModuleType("antenv.axon_hooks")
    mod._hook = None
    mod.set_axon_ntff_profile_hook = lambda h: setattr(mod, "_hook", h)
    mod.get_axon_ntff_profile_hook = lambda: mod._hook
    sys.modules["antenv.axon_hooks"] = mod
    antenv.axon_hooks = mod
    try:
        from trn_agent_boot.trn_boot import _ntff_profile_via_ctypes

        mod.set_axon_ntff_profile_hook(
            _ntff_profile_via_ctypes("/opt/axon/libaxon_pjrt.so")
        )
    except Exception:
        pass
    return mod._hook is not None


install_axon_hooks_shim()

E = 8  # experts == cores
D = 1024
H = 2048
TOP_K = 2

BF16 = mybir.dt.bfloat16
F32 = mybir.dt.float32

_CACHE: dict[int, object] = {}


def _route(x2d: np.ndarray, router_w: np.ndarray):
    """Float64 router. Returns per-expert token lists, per-expert combine
    weights, and for each token its (expert, slot-in-expert-batch) pairs."""
    T = x2d.shape[0]
    logits = x2d.astype(np.float64) @ router_w.astype(np.float64).T  # [T, E]
    order = np.argsort(-logits, axis=1, kind="stable")
    top2 = order[:, :TOP_K]  # [T, 2]
    lt = np.take_along_axis(logits, top2, axis=1)
    m = lt.max(axis=1, keepdims=True)
    ex = np.exp(lt - m)
    cw = (ex / ex.sum(axis=1, keepdims=True)).astype(np.float32)  # [T, 2]

    rows = []  # rows[e]: token ids routed to expert e (ascending)
    cw_e = []  # cw_e[e]: combine weight per routed token
    slot = np.empty((T, TOP_K), np.int64)  # slot[t, k]: row of t in expert batch
    for e in range(E):
        r = np.where((top2[:, 0] == e) | (top2[:, 1] == e))[0]
        k = np.where(top2[r, 0] == e, 0, 1)
        rows.append(r)
        cw_e.append(cw[r, k])
        slot[r, k] = np.arange(len(r))
    return rows, cw_e, top2, slot


def _build(C: int):
    """Build + compile the per-core Bass program for capacity C (mult of 128)."""
    assert C % 128 == 0
    nsub = C // 128  # token subtiles
    nc = bacc.Bacc("TRN2", target_bir_lowering=False, debug=False)

    xt = nc.declare_dram_parameter("xt", [D, C], BF16, isOutput=False)
    w1t = nc.declare_dram_parameter("w1t", [D, H], BF16, isOutput=False)
    w3t = nc.declare_dram_parameter("w3t", [D, H], BF16, isOutput=False)
    w2t = nc.declare_dram_parameter("w2t", [H, D], BF16, isOutput=False)
    cwt = nc.declare_dram_parameter("cwt", [128, nsub], F32, isOutput=False)
    y = nc.declare_dram_parameter("y", [C, D], F32, isOutput=True)

    xt_v = xt.rearrange("(a p) c -> p a c", p=128)  # [128, 8, C]
    w1_v = w1t.rearrange("(a p) h -> p a h", p=128)  # [128, 8, H]
    w3_v = w3t.rearrange("(a p) h -> p a h", p=128)
    w2_v = w2t.rearrange("(m p) d -> p m d", p=128)  # [128, 16, D]
    y_v = y.rearrange("(n p) d -> n p d", p=128)  # [nsub, 128, D]

    KA = D // 128  # 8 contraction chunks for matmul 1
    KM = H // 128  # 16 contraction chunks for matmul 2

    # token blocks: 512s + remainder (multiple of 128)
    blocks = []
    t0 = 0
    while t0 < C:
        tb = min(512, C - t0)
        blocks.append((t0, tb))
        t0 += tb

    with ExitStack() as ctx:
        tc = ctx.enter_context(tile.TileContext(nc))
        wpool = ctx.enter_context(tc.tile_pool(name="weights", bufs=1))
        xpool = ctx.enter_context(tc.tile_pool(name="x", bufs=2))
        hpool = ctx.enter_context(tc.tile_pool(name="h", bufs=2))
        spool = ctx.enter_context(tc.tile_pool(name="s", bufs=3))
        ypool = ctx.enter_context(tc.tile_pool(name="y", bufs=4))
        ppool = ctx.enter_context(tc.tile_pool(name="psum", bufs=2, space="PSUM"))

        # first token block's activations first — they gate the first matmul
        def xts_load(t0, tb):
            xa = xpool.tile([128, KA, tb], BF16, tag="xts")
            nc.sync.dma_start(xa[:], xt_v[:, :, t0 : t0 + tb])
            return xa

        xts0 = xts_load(0, blocks[0][1])

        # weights split into pieces (small first) so early matmuls start early
        PIECES = (1, 1, 2, 4, 4, 4)  # m-chunks per piece, sums to KM
        assert sum(PIECES) == KM
        w1p, w3p = [], []  # per m-chunk: (tile, offset)
        m0 = 0
        for p, sz in enumerate(PIECES):
            hs = slice(m0 * 128, (m0 + sz) * 128)
            t1 = wpool.tile([128, KA, sz * 128], BF16, tag=f"w1s{p}")
            nc.sync.dma_start(t1[:], w1_v[:, :, hs])
            t3 = wpool.tile([128, KA, sz * 128], BF16, tag=f"w3s{p}")
            nc.sync.dma_start(t3[:], w3_v[:, :, hs])
            for i in range(sz):
                w1p.append((t1, i))
                w3p.append((t3, i))
            m0 += sz

        w2p = []  # [m-half][d-half] tiles of [128, KM//2, 512]
        for mh in range(2):
            msl = slice(mh * (KM // 2), (mh + 1) * (KM // 2))
            row = []
            for dh in range(2):
                dsl = slice(dh * 512, (dh + 1) * 512)
                t2 = wpool.tile([128, KM // 2, 512], BF16, tag=f"w2s{mh}{dh}")
                nc.sync.dma_start(t2[:], w2_v[:, msl, dsl])
                row.append(t2)
            w2p.append(row)
        cws = wpool.tile([128, nsub], F32, tag="cws")
        nc.sync.dma_start(cws[:], cwt[:])

        for bi, (t0, tb) in enumerate(blocks):
            xts = xts0 if bi == 0 else xts_load(t0, tb)

            hts = hpool.tile([128, KM, tb], BF16, tag="hts")

            # phase A: h1T/h3T chunks + silu * mul -> hts
            for m in range(KM):
                w1s, o1 = w1p[m]
                w3s, o3 = w3p[m]
                ph1 = ppool.tile([128, tb], F32, tag="ph1")
                for a in range(KA):
                    nc.tensor.matmul(
                        ph1[:],
                        w1s[:, a, bass.ts(o1, 128)],
                        xts[:, a, :],
                        start=(a == 0),
                        stop=(a == KA - 1),
                    )
                ph3 = ppool.tile([128, tb], F32, tag="ph3")
                for a in range(KA):
                    nc.tensor.matmul(
                        ph3[:],
                        w3s[:, a, bass.ts(o3, 128)],
                        xts[:, a, :],
                        start=(a == 0),
                        stop=(a == KA - 1),
                    )
                sil = spool.tile([128, tb], BF16, tag="sil")
                nc.scalar.activation(
                    sil[:], ph1[:], mybir.ActivationFunctionType.Silu
                )
                nc.vector.tensor_mul(hts[:, m, :], sil[:], ph3[:])

            # phase B: y = hT.T @ w2T, scaled by cw
            for n in range(tb // 128):
                nsl = bass.ts(n, 128)
                gn = t0 // 128 + n  # global subtile index
                py0 = ppool.tile([128, 512], F32, tag="py0")
                py1 = ppool.tile([128, 512], F32, tag="py1")
                for m in range(KM):
                    mh, mr = divmod(m, KM // 2)
                    nc.tensor.matmul(
                        py0[:],
                        hts[:, m, nsl],
                        w2p[mh][0][:, mr, :],
                        start=(m == 0),
                        stop=(m == KM - 1),
                    )
                    nc.tensor.matmul(
                        py1[:],
                        hts[:, m, nsl],
                        w2p[mh][1][:, mr, :],
                        start=(m == 0),
                        stop=(m == KM - 1),
                    )
                ys0 = ypool.tile([128, 512], F32, tag="ys0")
                nc.vector.tensor_scalar_mul(ys0[:], py0[:], cws[:, gn : gn + 1])
                nc.sync.dma_start(y_v[gn][:, 0:512], ys0[:])
                ys1 = ypool.tile([128, 512], F32, tag="ys1")
                nc.vector.tensor_scalar_mul(ys1[:], py1[:], cws[:, gn : gn + 1])
                nc.sync.dma_start(y_v[gn][:, 512:1024], ys1[:])

    nc.compile()
    return nc


def _get(C: int):
    if C not in _CACHE:
        _CACHE[C] = _build(C)
    return _CACHE[C]


def _prepare_core_inputs(x2d, w1, w2, w3, rows, cw_e, C):
    bf = ml_dtypes.bfloat16
    nsub = C // 128
    in_maps = []
    for e in range(E):
        ce = len(rows[e])
        xt = np.zeros((D, C), bf)
        xt[:, :ce] = x2d[rows[e]].T.astype(bf)
        cwt = np.zeros((C,), np.float32)
        cwt[:ce] = cw_e[e]
        in_maps.append(
            {
                "xt": xt,
                "w1t": np.ascontiguousarray(w1[e].T.astype(bf)),  # [D, H]
                "w3t": np.ascontiguousarray(w3[e].T.astype(bf)),  # [D, H]
                "w2t": np.ascontiguousarray(w2[e].T.astype(bf)),  # [H, D]
                "cwt": np.ascontiguousarray(cwt.reshape(nsub, 128).T),
            }
        )
    return in_maps


def run(inputs: dict, trace: bool = False, trace_cores=None):
    """Core implementation; returns (output, BassKernelResults)."""
    x = np.asarray(inputs["x"])
    router_w = np.asarray(inputs["router_w"], np.float32)
    w1 = np.asarray(inputs["w1"], np.float32)
    w2 = np.asarray(inputs["w2"], np.float32)
    w3 = np.asarray(inputs["w3"], np.float32)

    B, S, _ = x.shape
    x2d = np.ascontiguousarray(x.reshape(-1, D).astype(np.float32))
    T = x2d.shape[0]

    rows, cw_e, top2, slot = _route(x2d, router_w)
    cmax = max(len(r) for r in rows)
    C = max(128, int(np.ceil(cmax / 128) * 128))

    nc = _get(C)
    in_maps = _prepare_core_inputs(x2d, w1, w2, w3, rows, cw_e, C)
    res = run_bass_kernel_spmd(
        nc,
        in_maps,
        list(range(E)),
        trace=trace,
        trace_cores=trace_cores,
    )

    Y = np.stack([res.results[e]["y"] for e in range(E)])  # [E, C, D] f32
    Yf = Y.reshape(E * C, D)
    fi = top2.astype(np.int64) * C + slot  # [T, 2]
    out = Yf[fi[:, 0]] + Yf[fi[:, 1]]
    return out.reshape(B, S, D).astype(x.dtype), res


def kernel(**inputs) -> np.ndarray:
    out, _ = run(inputs, trace=False)
    return out
